# revision 42
# baseline (speedup 1.0000x reference)
"""Trainium2 Bass kernel for the L2-normalized attention module.

Reference computation (per batch b):
    qkv = x @ w_qkv.T                        # [n, 3*dim]
    q,k,v per head h (d=64)                  # [n, d]
    dots = q @ k.T                           # [n, n]
    attn = dots / max(||dots_row||_2, eps) * g + b
    out_h = attn @ v                         # [n, d]
    final = concat_h(out_h) @ w_out.T + b_out

Key algebraic factorization: the score "nonlinearity" is only a per-row
scale r_i = 1/max(||dots_i||, eps), and ||dots_i||^2 = q_i^T (k^T k) q_i.
Therefore (with W = k^T v, G = k^T k):
    out_h^T[:, i] = (W^T q_i) / sqrt(q_i^T G q_i)
This removes the n x n score matrix entirely (8x fewer FLOPs).

Sharding: 8 cores = 2 batches x 4 head-groups (4 heads each).  Each core
computes its qkv slice, the factored attention for its 4 heads, and a
partial w_out projection; the host sums the 4 partials per batch and adds
b_out.  norm_g is folded into w_out on the host; norm_b (zero in practice)
is handled by an exact host-side rank-1 correction.

Schedule notes (from perfetto traces of earlier versions):
  - DMA priority order wq0, x0a, wq1, x0b, wkv, x1..x3, wo, bo, every
    transfer contiguous per partition.  PE warmup matmuls run off memset
    tiles (no DMA) from ~7us to open the HAM clock-gate; q0 starts as soon
    as wq-half + x0-half land (~11us).
  - The Gram [G|W] accumulates directly in PSUM across all kv chunks.
  - k is pre-scaled by 1/4 on the host (via w_k) so q*(Gq) fits fp16; the
    1/4^2 in G and 1/4 in W cancel exactly in (W^T q)/sqrt(q^T G q).
  - rinv is a single ACT Abs_reciprocal_sqrt (1/sqrt(|x|+eps^2)); its
    table set also contains copy, so the whole kernel runs on one
    activation table (Ln/Exp sit in different sets and would reload a
    1.3us table per chunk; DVE reciprocal is a ~4us ucode op).
  - phase B is software-pipelined with the remaining q chunks:
    q1 | b0 | q2 | b1 | C0 | q3 | t2,prods2 | t3,prods3 | o2,rep2 | C1 |
    o3,rep3 | C2 | C3.  The t/prods of chunks 2 and 3 are emitted before
    either chunk's rep/mult chain so the DVE queue never blocks a later
    chunk's prods behind an earlier chunk's outsc multiply.
"""

import numpy as np

from concourse import bacc
import concourse.mybir as mybir
import concourse.tile as tile
from concourse.bass_utils import run_bass_kernel_spmd

# Problem shape (hardcoded per contract)
B, N, DIM, H, D = 2, 2048, 1024, 16, 64
NCORES = 8
HPC = H // 4            # 4 heads per core
CH = 512                # sequence chunk (matmul moving free dim)
NCH = N // CH           # 4
KO = DIM // 128         # 8 contraction tiles for the projections
P = 128

F32 = mybir.dt.float32
F16 = mybir.dt.float16
MULT = mybir.AluOpType.mult
AFT = mybir.ActivationFunctionType


def _build_bass():
    nc = bacc.Bacc("TRN2", target_bir_lowering=False, debug=False)

    # eps^2 const for the Sqrt bias (mirrors Bass's const registration)
    _eps_t = nc.alloc_sbuf_tensor("const-float32-eps2", [128, 1], F32)
    nc.gpsimd.memset(_eps_t.ap(), 1e-24)
    nc.const_aps.aps[(F32, 1e-24)] = _eps_t.ap()

    x0_d = nc.dram_tensor("x0", [2, P, KO, 256], F16, kind="ExternalInput").ap()
    xr_d = nc.dram_tensor("xr", [NCH - 1, P, KO, CH], F16, kind="ExternalInput").ap()
    wq_d = nc.dram_tensor("wq", [2, P, KO, 128], F16, kind="ExternalInput").ap()
    wkv_d = nc.dram_tensor("wkv", [P, KO, 512], F16, kind="ExternalInput").ap()
    wo_d = nc.dram_tensor("wo", [P, 2, 1024], F16, kind="ExternalInput").ap()
    bo_d = nc.dram_tensor("bo", [P, 128], F16, kind="ExternalInput").ap()
    out_d = nc.dram_tensor("outT", [8, P, N], F16, kind="ExternalOutput").ap()

    with tile.TileContext(nc) as tc:
        with (
            tc.tile_pool(name="w", bufs=1) as wpool,
            tc.tile_pool(name="big", bufs=1) as bigpool,
            tc.tile_pool(name="small", bufs=4) as smallpool,
            tc.tile_pool(name="stage", bufs=4) as stagepool,
            tc.tile_pool(name="pa", bufs=3, space="PSUM") as pa,
            tc.tile_pool(name="pg", bufs=2, space="PSUM") as pg,
            tc.tile_pool(name="pc", bufs=3, space="PSUM") as pc,
        ):
            # ---- persistent tiles ----
            qT = bigpool.tile([P, 2, N], F16, name="qT_sb")        # [d-pair, n]
            kv = bigpool.tile([P, 16, CH], F16, name="kv_sb")      # [n-tile, [ke|ko|ve|vo]]

            # gw tiles are zeroed on-device (no DMA) so the PE warmup can
            # start immediately and open the HAM clock-gate during the DMA
            # fill; warmup results are discarded.
            gwG_all = wpool.tile([P, 2, 128], F16, name="gwG_all")
            nc.gpsimd.memset(gwG_all, 0.0)
            gwW_all = wpool.tile([P, 2, 128], F16, name="gwW_all")
            nc.gpsimd.memset(gwW_all, 0.0)
            warm_sb = smallpool.tile([P, 4], F32, name="warm_sb")
            ps_warm = pa.tile([P, 128], F32, tag="pa", name="ps_warm")
            for i in range(40):
                nc.tensor.matmul(ps_warm, lhsT=gwG_all[:, 0, :],
                                 rhs=gwW_all[:, 0, :], start=True, stop=True)
            nc.vector.tensor_copy(warm_sb, ps_warm[:, 0:4])

            # Input DMAs in priority order; every transfer is contiguous
            # per partition.
            wq = wpool.tile([P, 2, KO, 128], F16, name="wq_sb")
            nc.sync.dma_start(wq[:, 0], wq_d[0])
            x0 = wpool.tile([P, 2, KO, 256], F16, name="x0_sb")
            nc.sync.dma_start(x0[:, 0], x0_d[0])
            nc.sync.dma_start(wq[:, 1], wq_d[1])
            nc.sync.dma_start(x0[:, 1], x0_d[1])
            wkv = wpool.tile([P, KO, 512], F16, name="wkv_sb")
            nc.sync.dma_start(wkv, wkv_d)
            xr = wpool.tile([P, NCH - 1, KO, CH], F16, name="xr_sb")
            for cc in range(NCH - 1):
                nc.sync.dma_start(xr[:, cc, :, :], xr_d[cc])
            wo = wpool.tile([P, 2, 1024], F16, name="wo_sb")
            nc.sync.dma_start(wo, wo_d)
            bo = wpool.tile([P, 128], F16, name="bo_sb")
            nc.sync.dma_start(bo, bo_d)

            def x_lhsT(c, ko, nt):
                # x^T tile [128 dims, 128 tokens] for chunk c, token tile nt
                if c == 0:
                    return x0[:, nt // 2, ko, (nt % 2) * 128:(nt % 2) * 128 + 128]
                return xr[:, c - 1, ko, nt * 128:(nt + 1) * 128]

            def x_rhs(c, ko):
                # x^T chunk [128 dims, 512 tokens] (chunks 1..3 only)
                return xr[:, c - 1, ko, :]

            # ---- phase A ----
            # q0 first (needs only wq + x0), as half-chunk groups so it can
            # start the moment the first two transfers land; its four
            # groups exactly fill the PE while wkv streams in.
            for hc in range(2):
                hs = slice(hc * 256, (hc + 1) * 256)
                for mt in range(2):
                    ps_q = pa.tile([P, 256], F32, tag="pa", name="ps_q0")
                    for ko in range(KO):
                        nc.tensor.matmul(
                            ps_q,
                            lhsT=(wq[:, mt, ko, :]),
                            rhs=(x0[:, hc, ko, :]),
                            start=(ko == 0),
                            stop=(ko == KO - 1),
                        )
                    nc.vector.tensor_copy(qT[:, mt, hs], ps_q)

            # kv chunks + incremental Gram: [G|W] per pair accumulates in
            # PSUM across ALL chunks (single accumulation group), so no
            # DVE adds are needed.
            gw_ps = [
                pg.tile([P, 256], F32, tag="pg", name=f"gw_ps{p}")
                for p in range(2)
            ]
            # The gram batch for chunk c is emitted one chunk late (after
            # chunk c+1's first kv group): the kv groups then run
            # back-to-back with hidden LDWEIGHTS, and the batched gram
            # matmuls amortize their weight loads instead of paying the
            # group-restart exposure every token tile.
            def gram_batch(c, first, last):
                for nt in range(4):
                    jt = c * 4 + nt
                    for p in range(2):
                        blk = kv[:, jt, p * 256:(p + 1) * 256]
                        nc.tensor.matmul(
                            gw_ps[p], lhsT=blk[:, 0:128], rhs=blk,
                            start=(first and nt == 0),
                            stop=(last and nt == 3),
                        )

            for c in range(NCH):
                for nt in range(4):
                    jt = c * 4 + nt
                    ps_kv = pa.tile([P, CH], F32, tag="pa", name="ps_kv")
                    for ko in range(KO):
                        nc.tensor.matmul(
                            ps_kv,
                            lhsT=x_lhsT(c, ko, nt),
                            rhs=(wkv[:, ko, :]),
                            start=(ko == 0),
                            stop=(ko == KO - 1),
                        )
                    if nt % 2 == 0:
                        nc.scalar.copy(kv[:, jt, :], ps_kv)
                    else:
                        nc.vector.tensor_copy(kv[:, jt, :], ps_kv)
                    if c > 0 and nt == 0:
                        gram_batch(c - 1, first=(c == 1), last=False)
                    if c == NCH - 1:
                        # the last chunk's grams interleave per token tile
                        # so the accumulation stop (which gates the gw
                        # copies and all of phase B) lands right after the
                        # final kv copy instead of a trailing batch.
                        for p in range(2):
                            blk = kv[:, jt, p * 256:(p + 1) * 256]
                            nc.tensor.matmul(
                                gw_ps[p], lhsT=blk[:, 0:128], rhs=blk,
                                start=False, stop=(nt == 3),
                            )

            # block-diagonal lhsT tiles (two heads stacked on K=128), read
            # straight out of the Gram PSUM banks.
            # split DVE/ACT so the 8 small copies drain in parallel; the
            # B pipeline's first t matmuls wait on these.
            for p in range(2):
                nc.vector.tensor_copy(gwG_all[0:64, p, 0:64], gw_ps[p][0:64, 0:64])
                nc.vector.tensor_copy(gwG_all[64:128, p, 64:128], gw_ps[p][64:128, 64:128])
                nc.scalar.copy(gwW_all[0:64, p, 0:64], gw_ps[p][0:64, 128:192])
                nc.scalar.copy(gwW_all[64:128, p, 64:128], gw_ps[p][64:128, 192:256])

            # ---- phase B, software-pipelined with q1..q3 ----
            def q_chunk(c):
                cs = slice(c * CH, (c + 1) * CH)
                for mt in range(2):
                    ps_q = pa.tile([P, CH], F32, tag="pa", name="ps_q")
                    for ko in range(KO):
                        nc.tensor.matmul(
                            ps_q,
                            lhsT=(wq[:, mt, ko, :]),
                            rhs=x_rhs(c, ko),
                            start=(ko == 0),
                            stop=(ko == KO - 1),
                        )
                    nc.vector.tensor_copy(qT[:, mt, cs], ps_q)

            osc = {}
            bstate = {}

            def b_front(c):
                # t, o matmuls + prods; keeps the DVE queue free of the
                # rep/mult chain so a later chunk's prods are not blocked
                # head-of-line behind an earlier chunk's outsc mult.
                cs = slice(c * CH, (c + 1) * CH)
                ps_t = {}
                for p in range(2):
                    ps_t[p] = pa.tile([P, CH], F32, tag="pa", name="ps_t")
                    nc.tensor.matmul(ps_t[p], lhsT=(gwG_all[:, p, :]),
                                     rhs=(qT[:, p, cs]), start=True, stop=True)
                prods = {}
                for p in range(2):
                    prods[p] = stagepool.tile([P, CH], F16, name="prod",
                                              tag="prod", bufs=4)
                    nc.vector.tensor_tensor(prods[p], ps_t[p], qT[:, p, cs], MULT)
                bstate[c] = prods

            def b_back(c):
                # rinv = 1/sqrt(|norm2| + 1e-24) in ONE ACT op; the
                # abs_reciprocal_sqrt table set also contains copy, so the
                # whole kernel uses a single activation table.
                cs = slice(c * CH, (c + 1) * CH)
                prods = bstate.pop(c)
                ps_o = {}
                for p in range(2):
                    ps_o[p] = pg.tile([P, CH], F32, tag="pg", name="ps_o")
                    nc.tensor.matmul(ps_o[p], lhsT=(gwW_all[:, p, :]),
                                     rhs=(qT[:, p, cs]), start=True, stop=True)
                for p in range(2):
                    ps_rep = pa.tile([P, CH], F32, tag="pa", name="ps_rep")
                    nc.tensor.matmul(ps_rep, lhsT=(bo), rhs=(prods[p]),
                                     start=True, stop=True)
                    s = stagepool.tile([P, CH], F32, name="s", tag="s", bufs=4)
                    nc.scalar.activation(s, ps_rep, AFT.Abs_reciprocal_sqrt,
                                         bias=1e-24)
                    o = stagepool.tile([P, CH], F16, name="osc", tag="osc",
                                       bufs=4)
                    nc.vector.tensor_tensor(o, ps_o[p], s, MULT)
                    osc[p, c] = o

            def b_head(c):
                b_front(c)
                b_back(c)

            def c_chunk(c):
                cs = slice(c * CH, (c + 1) * CH)
                for mt in range(8):
                    # the last chunk's final two groups borrow the pg
                    # banks (idle once mult3 is done) so the tail runs
                    # with a 5-deep rotation and never waits on a stage.
                    if c == NCH - 1 and mt >= 6:
                        ps_f = pg.tile([P, CH], F32, tag="pg", name="ps_f")
                    else:
                        ps_f = pc.tile([P, CH], F32, tag="pc", name="ps_f")
                    for kt in range(2):
                        nc.tensor.matmul(
                            ps_f,
                            lhsT=(wo[:, kt, mt * 128:(mt + 1) * 128]),
                            rhs=(osc[kt, c]),
                            start=(kt == 0),
                            stop=(kt == 1),
                        )
                    st = stagepool.tile([P, CH], F16, name="st", tag="st",
                                        bufs=8)
                    if mt % 2 == 0:
                        nc.vector.tensor_copy(st, ps_f)
                        nc.sync.dma_start(out_d[mt, :, cs], st)
                    else:
                        # ACT stages issue their own DMA (ACT is a HWDGE
                        # engine) so trailing output DMAs drain on two
                        # queues in parallel.
                        nc.scalar.copy(st, ps_f)
                        nc.scalar.dma_start(out_d[mt, :, cs], st)

            q_chunk(1)
            b_head(0)
            q_chunk(2)
            b_head(1)
            c_chunk(0)
            q_chunk(3)
            b_front(2)
            b_front(3)
            b_back(2)
            c_chunk(1)
            b_back(3)
            c_chunk(2)
            c_chunk(3)

    nc.compile()
    return nc


_NC_CACHE = None


def _get_nc():
    global _NC_CACHE
    if _NC_CACHE is None:
        _NC_CACHE = _build_bass()
    return _NC_CACHE


def _build_in_maps(x, w_qkv, w_out_g):
    """Per-core device inputs (shared NEFF, different shards)."""
    bo = np.zeros((P, 128), np.float16)
    bo[0:64, 0:64] = 1.0
    bo[64:128, 64:128] = 1.0

    in_maps = []
    for core in range(NCORES):
        bi = core // 4
        hg = core % 4
        # x^T tiled chunk-major; chunk 0 split into two contiguous halves
        xt0 = x[bi].T.reshape(KO, P, N).transpose(1, 0, 2)  # [p, ko, n]
        x0 = np.ascontiguousarray(
            np.stack([xt0[:, :, 0:256], xt0[:, :, 256:512]]))
        xr = np.ascontiguousarray(
            np.stack([xt0[:, :, cc * CH:(cc + 1) * CH] for cc in range(1, NCH)]))
        # q rows of this head group, transposed, split into mt halves
        rows_q = slice(hg * 256, hg * 256 + 256)
        wqf = w_qkv[rows_q].T.reshape(KO, P, 256).transpose(1, 0, 2)  # [p,ko,256]
        wq = np.ascontiguousarray(
            np.stack([wqf[:, :, 0:128], wqf[:, :, 128:256]]))  # [2,p,ko,128]
        # per-head-pair [k_even | k_odd | v_even | v_odd] blocks.  k is
        # scaled by 1/4 so q*(Gq) stays in fp16 range; the 1/16 in G and
        # 1/4 in W cancel exactly in (W^T q) / sqrt(q^T G q).
        blocks = []
        for pp in range(2):
            he = hg * HPC + 2 * pp
            ho = he + 1
            blocks.append(0.25 * w_qkv[DIM + he * D: DIM + (he + 1) * D])
            blocks.append(0.25 * w_qkv[DIM + ho * D: DIM + (ho + 1) * D])
            blocks.append(w_qkv[2 * DIM + he * D: 2 * DIM + (he + 1) * D])
            blocks.append(w_qkv[2 * DIM + ho * D: 2 * DIM + (ho + 1) * D])
        wkv_local = np.concatenate(blocks, axis=0)  # [512, dim]
        wkv = np.ascontiguousarray(
            wkv_local.T.reshape(KO, P, 512).transpose(1, 0, 2))
        # w_out columns for this head group (norm_g folded), transposed
        wo_local = w_out_g[:, hg * 256:(hg + 1) * 256]  # [1024, 256]
        wo = np.ascontiguousarray(
            wo_local.T.reshape(2, P, 1024).transpose(1, 0, 2))
        in_maps.append({
            "x0": x0.astype(np.float16), "xr": xr.astype(np.float16),
            "wq": wq.astype(np.float16),
            "wkv": wkv.astype(np.float16), "wo": wo.astype(np.float16),
            "bo": bo,
        })
    return in_maps


def kernel(x, w_qkv, w_out, b_out, norm_g, norm_b):
    x = np.ascontiguousarray(np.asarray(x, dtype=np.float32))
    w_qkv = np.asarray(w_qkv, dtype=np.float32)
    w_out = np.asarray(w_out, dtype=np.float32)
    b_out = np.asarray(b_out, dtype=np.float32)
    g = np.asarray(norm_g, dtype=np.float32).reshape(H)
    bb = np.asarray(norm_b, dtype=np.float32).reshape(H)

    # Fold norm_g into w_out columns (attn scale per head passes through @v).
    w_out_g = w_out.copy()
    for h in range(H):
        w_out_g[:, h * D:(h + 1) * D] *= g[h]

    in_maps = _build_in_maps(x, w_qkv, w_out_g)

    nc = _get_nc()
    res = None
    last_exc = None
    for _attempt in range(3):
        try:
            res = run_bass_kernel_spmd(nc, in_maps, core_ids=list(range(NCORES)))
            break
        except Exception as e:  # transient NRT_EXEC_UNIT_UNRECOVERABLE etc.
            last_exc = e
            import time as _time
            _time.sleep(5)
    if res is None:
        raise last_exc

    out = np.zeros((B, N, DIM), np.float32)
    for core in range(NCORES):
        bi = core // 4
        partial = res.results[core]["outT"].reshape(DIM, N).astype(np.float32)
        out[bi] += partial.T
    out += b_out[None, None, :]

    # Exact rank-1 correction for norm_b (zero in practice).
    if np.any(bb != 0.0):
        for bi in range(B):
            corr = np.zeros(DIM, np.float64)
            for h in range(H):
                wv = w_qkv[2 * DIM + h * D: 2 * DIM + (h + 1) * D]  # [d, dim]
                vsum = (x[bi].astype(np.float64) @ wv.T.astype(np.float64)).sum(axis=0)
                # the +b term bypasses the g scale, so use the raw w_out
                corr += bb[h] * (w_out[:, h * D:(h + 1) * D].astype(np.float64) @ vsum)
            out[bi] += corr.astype(np.float32)[None, :]

    return out
